# revision 51
# baseline (speedup 1.0000x reference)
"""NT-Xent / SimCLR contrastive loss on 8 Trainium2 NeuronCores.

Symmetry-halved data-parallel strategy:
  sim is symmetric, so each unordered row pair only needs one on-device
  exp(). Core i (inputs rolled by -1024*i rows) computes its 1024 rows
  against rolled columns [0, 5120) only:
    d0 = cols [0,1024):    self block (complete rowsums, fully covers its
                           own pairs - both (r,c) and (c,r) are inside).
    d1-d3 = [1024,4096):   each pair {i,i+d} computed ONCE (by core i);
                           core i+d's missing mirror contributions are
                           returned as per-column sums of exp (colsums) and
                           added on the host.
    d4 = [4096,5120):      pair {i,i+4} computed by BOTH cores (cheaper
                           than exchanging mirrors); positives live on this
                           block's diagonal.
  Device pipeline per core (fp8 DoubleRow, one kernel launch):
    phase A (per 1024-row half): load bf16 rows; 1/||row|| via fused
      square+rowsum then a Quake-style bitwise rsqrt + 2 Newton steps (all
      DVE - keeps ACT free and avoids cross-queue stalls); scale rows into
      permuted fp8e4 staging; DRAM scratch store + xbar transpose-load into
      [128,1024] u16 repsT8 tiles (u16 = the 2 DoubleRow fp8 planes).
    phase B (chunk-outer, m-inner; chunks [1024,1536,1536,1024] wide):
      sim chunk via DR fp8 matmuls (kk-outer so LDWEIGHTS amortizes over
      the chunk) -> PSUM [128,<=1536]; ACT exp(2/C^2 * sim) with fused
      row-sum accumulate; colsums of the d1-d3 exp tiles via ones-onehot
      matmuls accumulated over m in a [8,512] PSUM bank; positives diag
      extracted from the d4 chunk with an identity-mask STT (DVE).
    e_self is NOT extracted: sim_ii = 1 +- ~0.4% in fp8, so the host
    subtracts exp(2) exactly - error ~1e-6 relative on the loss.
  Host: assemble denom[8192] from per-core partial rowsums + mirrored
  colsums (undoing the transpose permutation), then
  loss = mean(ln(denom) - ln(e_pos)).
"""

import math
import sys
import threading
from unittest import mock

sys.path.insert(0, "/opt/trn_rl_repo")

import numpy as np  # noqa: E402
import ml_dtypes  # noqa: E402

import concourse.tile as tile  # noqa: E402
from concourse import bacc, mybir  # noqa: E402
from concourse.bass_utils import run_bass_kernel_spmd  # noqa: E402
from concourse.hw_specs import get_activation_tables  # noqa: E402
from concourse.masks import make_identity  # noqa: E402
from contextlib import ExitStack  # noqa: E402

P = 128
D = 512
TWO_N = 8192
N_CORES = 8
ROWS_PER_CORE = TWO_N // N_CORES  # 1024
T_INV = 2.0  # 1 / temperature (0.5)

HH = 5  # 1024-row halves normalized/transposed per core (cols [0, 5120))
HC = 1024  # columns per half
TPH = HC // P  # 8 [128, 512] row tiles per half
MB = ROWS_PER_CORE // P  # 8 m-blocks of 128 rows per core
KK = 2  # DoubleRow packed K chunks (256 features each)
NSLOT = 3  # phase-B chunks per m (rowsum slots)
PSW = 2048  # PSUM sim tile width (4 banks x 2 bufs = all 8; colsums ride
            # the same ring as disjoint [1,512] slices)
MAGIC = 0x5F3759DF  # fast inverse sqrt

C_SCALE = 512.0  # fp8 range scale; rows stored as C*x/||x||
SIM_SCALE = T_INV / (C_SCALE * C_SCALE)  # exact power of two: 2^-17
E_SELF = float(np.exp(np.float64(T_INV)))  # host-subtracted self term

# phase-B chunks: (col0, width). Chunk 0 only needs half 0 (short serial
# prefix); positives (d4) sit in chunk 2's second half.
CHUNKS = [
    (0, 1024),     # d0 (self block)
    (1024, 2048),  # d1 + d2
    (3072, 2048),  # d3 + d4
]
# colsum classes: chunk 1 contributes 4 512-wide pairs (d1a,b d2a,b),
# chunk 2 contributes 2 (d3a,b; d4 needs no mirror). Host mapping of pair
# c: half 1 + c//2, within-half columns (c%2)*512 ..
ACC_W = {1: 2048, 2: 1024}  # colsum-relevant width of each chunk's et

FP32 = mybir.dt.float32
BF16 = mybir.dt.bfloat16
FP8 = mybir.dt.float8e4
U16 = mybir.dt.uint16
I32 = mybir.dt.int32
AF = mybir.ActivationFunctionType
ALU = mybir.AluOpType
AX = mybir.AxisListType
DR = mybir.MatmulPerfMode.DoubleRow


def _filtered_activation_tables(arch):
    """Steer every Exp/Ln/Copy activation to the one table set containing
    both Exp and Ln, so the table-load pass cannot thrash between sets."""
    tables = get_activation_tables(arch)
    target = None
    for name, funcs in tables.items():
        if AF.Exp in funcs and AF.Ln in funcs:
            target = name
            break
    if target is None:
        return tables
    steer = {AF.Exp, AF.Ln, AF.Copy, AF.Identity}
    return {
        name: (funcs if name == target else funcs - steer)
        for name, funcs in tables.items()
    }


def _build_kernel():
    nc = bacc.Bacc("TRN2", target_bir_lowering=False, debug=False,
                   num_devices=N_CORES)
    reps = nc.dram_tensor("reps", [HH * HC, D], BF16,
                          kind="ExternalInput").ap()
    rs_out = nc.dram_tensor("rs_out", [P, MB * NSLOT], FP32,
                            kind="ExternalOutput").ap()
    pos_out = nc.dram_tensor("pos_out", [P, MB], FP32,
                             kind="ExternalOutput").ap()
    cs_out = nc.dram_tensor("cs_out", [1, 3072], FP32,
                            kind="ExternalOutput").ap()

    with tile.TileContext(nc) as tc, ExitStack() as ctx:
        rows_pool = ctx.enter_context(tc.tile_pool(name="rows", bufs=1))
        stag_pool = ctx.enter_context(tc.tile_pool(name="stag", bufs=3))
        sq_pool = ctx.enter_context(tc.tile_pool(name="sq", bufs=2))
        stats_pool = ctx.enter_context(tc.tile_pool(name="stats", bufs=1))
        repsT_pool = ctx.enter_context(tc.tile_pool(name="repsT", bufs=1))
        repsT0_pool = ctx.enter_context(tc.tile_pool(name="repsT0", bufs=1))
        dram_pool = ctx.enter_context(
            tc.tile_pool(name="scratch", bufs=KK * HH, space="DRAM"))
        psum_pool = ctx.enter_context(
            tc.tile_pool(name="psum", bufs=2, space="PSUM"))
        exp_pool = ctx.enter_context(tc.tile_pool(name="exp", bufs=10))
        junk_pool = ctx.enter_context(tc.tile_pool(name="junk", bufs=2))
        epi_pool = ctx.enter_context(tc.tile_pool(name="epi", bufs=1))

        # --- constants -----------------------------------------------------
        ident = stats_pool.tile([P, P], FP32, tag="ident", name="ident")
        make_identity(nc, ident[:])
        ones = stats_pool.tile([P, 1], BF16, tag="ones", name="ones")
        nc.gpsimd.memset(ones[:], 1.0)

        # accumulators
        rs_all = stats_pool.tile([P, MB * NSLOT], FP32, tag="rs",
                                 name="rs_all")
        pos = stats_pool.tile([P, MB], FP32, tag="pos", name="pos")
        cs_sb = epi_pool.tile([1, 3072], FP32, tag="cssb", name="cs_sb")

        # repsT8[kk][hh]: [128, 1024] u16 - partition p = feature pair
        # (kk*256 + 2p, 2p+1) packed as 2 fp8 bytes; column q = half row
        # (q%8)*128 + q//8 (scratch permutation). fp8 bitcast views give
        # the DoubleRow [128, 2, N] moving-operand APs directly.
        repsT8 = [[repsT_pool.tile([P, HC], U16, tag=f"rT{kk}_{hh}",
                                   name=f"repsT8_{kk}_{hh}")
                   for hh in range(HH)]
                  for kk in range(KK)]

        def rhs_ap(kk, hh, sub):
            v = repsT8[kk][hh][:].bitcast(FP8).rearrange(
                "p (n two) -> p two n", two=2)
            return v[:, :, sub * 512:(sub + 1) * 512]

        # repsT0[kk]: [128, 2*1024] fp8, plane-slab layout - LDWEIGHTS
        # rejects the byte-interleaved stride-2 AP, so the core's own 1024
        # columns are deinterleaved for the stationary side.
        repsT0 = [repsT0_pool.tile([P, 2 * ROWS_PER_CORE], FP8,
                                   tag=f"rT0_{kk}", name=f"repsT0_{kk}")
                  for kk in range(KK)]

        def lhs_ap(kk, m):
            v = repsT0[kk][:].rearrange("p (two m) -> p two m", two=2)
            return v[:, :, m * P:(m + 1) * P]

        # --- loads ---------------------------------------------------------
        # half 0 lands in four quarter tiles so the first square pass only
        # waits for 256 rows (dependency tracking is tile-granular).
        rows_q0 = [rows_pool.tile([P, 2 * D], BF16, tag=f"rows0q{qq}",
                                  name=f"rows_0q{qq}") for qq in range(4)]
        rows_t = [None] + [rows_pool.tile([P, TPH * D], BF16,
                                          tag=f"rows{hh}", name=f"rows_{hh}")
                           for hh in range(1, HH)]

        def row_slice(hh, tl):
            if hh == 0:
                return rows_q0[tl // 2][:, (tl % 2) * D:(tl % 2 + 1) * D]
            return rows_t[hh][:, tl * D:(tl + 1) * D]

        def issue_load(hh):
            if hh == 0:
                for qq in range(4):
                    r0 = qq * 2 * P
                    src = reps[r0:r0 + 2 * P, :].rearrange(
                        "(t p) d -> p t d", p=P)
                    nc.sync.dma_start(
                        out=rows_q0[qq][:].rearrange("p (t d) -> p t d", d=D),
                        in_=src)
                return
            for s in range(2):
                r0 = hh * HC + s * (TPH // 2) * P
                src = reps[r0:r0 + (TPH // 2) * P, :].rearrange(
                    "(t p) d -> p t d", p=P)
                dst = rows_t[hh][:, s * (TPH // 2) * D:(s + 1) * (TPH // 2) *
                                 D].rearrange("p (t d) -> p t d", d=D)
                nc.sync.dma_start(out=dst, in_=src)

        def phase_a(hh):
            n2 = stats_pool.tile([P, TPH], FP32, tag="n2", bufs=3,
                                 name=f"n2_{hh}")
            for tl in range(TPH):
                sq = sq_pool.tile([P, D], BF16, tag="sq",
                                  name=f"sq_{hh}_{tl}")
                rt = row_slice(hh, tl)
                nc.vector.scalar_tensor_tensor(
                    out=sq[:], in0=rt, scalar=1.0, in1=rt,
                    op0=ALU.mult, op1=ALU.mult,
                    accum_out=n2[:, tl:tl + 1])
            # inv = C * rsqrt(n2), entirely on DVE (bit trick + 1 Newton
            # step, max rel err ~1.6e-3 on inv -> ~1e-4 on the loss): keeps
            # ACT for exps/copies and avoids cross-queue ping-pong stalls.
            # (Tried computing half 0's inv on the prefix-idle ACT via
            # ln/exp instead: consistently ~3us SLOWER end-to-end.)
            w = stats_pool.tile([P, 4 * TPH], FP32, tag="rsq", bufs=3,
                                name=f"rsq_{hh}")
            t_i, y_i = w[:, 0:TPH].bitcast(I32), w[:, TPH:2 * TPH]
            aa, cc = w[:, 2 * TPH:3 * TPH], w[:, 3 * TPH:4 * TPH]
            inv = stats_pool.tile([P, TPH], FP32, tag="inv", bufs=3,
                                  name=f"inv_{hh}")
            nc.vector.tensor_scalar(out=t_i, in0=n2[:].bitcast(I32),
                                    scalar1=1, scalar2=None,
                                    op0=ALU.logical_shift_right)
            nc.vector.tensor_scalar(out=y_i.bitcast(I32), in0=t_i,
                                    scalar1=MAGIC, scalar2=-1,
                                    op0=ALU.subtract, op1=ALU.mult)
            nc.vector.tensor_mul(aa, y_i, y_i)       # y0^2
            nc.vector.scalar_tensor_tensor(
                out=cc, in0=aa, scalar=-0.5, in1=n2[:],
                op0=ALU.mult, op1=ALU.mult)          # -0.5*n2*y0^2
            nc.vector.tensor_scalar(out=aa, in0=cc, scalar1=1.5,
                                    scalar2=C_SCALE, op0=ALU.add,
                                    op1=ALU.mult)    # C*(1.5 - n2*y0^2/2)
            nc.vector.tensor_mul(inv[:], y_i, aa)    # C * rsqrt(n2)
            # permuted fp8 staging: byte addr = kk*2048 + t*256 + c so each
            # kk store is one contiguous 2 KiB run per partition.
            stag = stag_pool.tile([P, KK * TPH * 256], FP8, tag="stag",
                                  name=f"stag_{hh}")
            sview = stag[:].rearrange("p (kk t c) -> p t kk c",
                                      kk=KK, c=256)
            for tl in range(TPH):
                src3 = row_slice(hh, tl).rearrange(
                    "p (kk c) -> p kk c", kk=KK)
                nc.vector.tensor_scalar_mul(
                    sview[:, tl], src3, inv[:, tl:tl + 1])
            # scratch row q = p*8 + t holds half row t*128 + p; the
            # transpose-load is then fully contiguous.
            for kk in range(KK):
                scr = dram_pool.tile([HC, P], U16, tag=f"scr{kk}_{hh}",
                                     name=f"scr_{kk}_{hh}")
                src = stag[:, kk * TPH * 256:(kk + 1) * TPH * 256]
                nc.sync.dma_start(
                    out=scr[:].rearrange("(p t) c -> p t c", p=P),
                    in_=src.bitcast(U16).rearrange("p (t c) -> p t c", c=P))
                nc.sync.dma_start_transpose(repsT8[kk][hh][:], scr[:])
            # prefetch AFTER this half's stores/transposes: the sync DMA
            # queue is in-order, so loads issued earlier would delay them.
            if hh + 2 < HH:
                issue_load(hh + 2)

        # permuted column q of an et tile (within a half) holds half row
        # (q%8)*128 + q//8, so the columns for m-block rows m*128+j sit at
        # positions 8*j + m.
        def colsel(ap_1024, m):
            return ap_1024.rearrange("p (j s) -> p s j", s=TPH)[:, m, :]

        def emit_copies(s0, s1):
            # deinterleave + unpermute the core's own 1024 columns into
            # plane-slab lhsT (fp8 index 2*(8j+s)+i for half row s*128+j).
            # ACT does these strided copies (~0.7-1.6us each; DVE/GPSIMD
            # measured ~4us on the same APs).
            for kk in range(KK):
                iv = repsT8[kk][0][:].bitcast(FP8).rearrange(
                    "p (j s two) -> p two s j", two=2, s=TPH)
                ov = repsT0[kk][:].rearrange(
                    "p (two s m) -> p two s m", two=2, s=MB)
                for i in range(2):
                    nc.scalar.activation(
                        ov[:, i, s0:s1], iv[:, i, s0:s1], AF.Copy)

        # colsum accumulators: acc[ci] = sum over m of et tiles, built by
        # DVE bf16 adds in its post-phase-A idle window; the PE then only
        # streams one ones-matmul per 512-wide pair instead of eight.
        acc_t = {ci: stats_pool.tile([P, ACC_W[ci]], BF16, tag=f"acc{ci}",
                                     name=f"acc_{ci}")
                 for ci in ACC_W}

        def phase_b(ci, m0, m1):
            col0, w = CHUNKS[ci]
            subs = [((col0 + 512 * s) // HC, ((col0 + 512 * s) % HC) // 512)
                    for s in range(w // 512)]
            for m in range(m0, m1):
                ps = psum_pool.tile([P, PSW], FP32, tag="ps",
                                    name=f"ps_{ci}_{m}")
                for kk in range(KK):
                    for idx, (hh, sub) in enumerate(subs):
                        nc.tensor.matmul(
                            ps[:, idx * 512:(idx + 1) * 512],
                            lhsT=lhs_ap(kk, m),
                            rhs=rhs_ap(kk, hh, sub),
                            start=(kk == 0), stop=(kk == KK - 1),
                            perf_mode=DR)
                et = exp_pool.tile([P, PSW], BF16, tag="et",
                                   name=f"et_{ci}_{m}")
                sl = m * NSLOT + ci
                nc.scalar.activation(
                    et[:, :w], ps[:, :w], AF.Exp, scale=SIM_SCALE,
                    accum_out=rs_all[:, sl:sl + 1])
                if ci in acc_t:
                    a = acc_t[ci]
                    aw = ACC_W[ci]
                    if m == 0:
                        nc.vector.tensor_copy(a[:], et[:, :aw])
                    else:
                        nc.vector.tensor_add(a[:], a[:], et[:, :aw])
                if ci == 2:
                    # positives: global col 4096 + (m*128+j) at permuted
                    # position 8j+m of the d4 half (= this chunk's second
                    # half). exp(2*s_pos) read from et; ln on the host.
                    junk = junk_pool.tile([P, P], FP32, tag="junk",
                                          name=f"junk_p_{m}")
                    nc.vector.scalar_tensor_tensor(
                        out=junk[:], in0=colsel(et[:, HC:2 * HC], m),
                        scalar=1.0, in1=ident[:],
                        op0=ALU.mult, op1=ALU.mult,
                        accum_out=pos[:, m:m + 1])

        def emit_cs(ci, sb_off):
            # colsums land in disjoint [1,512] slices of one ps-ring tile
            # (ones-lhsT matmuls), then a DVE drain into cs_sb. Emitted
            # late enough that the PE reaches these only after the DVE
            # finished summing the class's et tiles.
            aw = ACC_W[ci]
            pst = psum_pool.tile([P, PSW], FP32, tag="ps",
                                 name=f"cs_ps_{ci}")
            for s in range(aw // 512):
                nc.tensor.matmul(
                    pst[:1, s * 512:(s + 1) * 512], lhsT=ones[:],
                    rhs=acc_t[ci][:, s * 512:(s + 1) * 512],
                    start=True, stop=True)
            if ci == 2:
                # the final drain is on the tail critical path: ACT is done
                # by then while the DVE still has the last pos-diag STT, so
                # run it on ACT to overlap.
                nc.scalar.activation(cs_sb[:, sb_off:sb_off + aw],
                                     pst[:1, :aw], AF.Copy)
            else:
                nc.vector.tensor_copy(cs_sb[:, sb_off:sb_off + aw],
                                      pst[:1, :aw])

        # --- pipeline ------------------------------------------------------
        # Emission order = per-engine queue order (queues execute in order,
        # so anything emitted early that waits on a late dependency blocks
        # its whole queue). phase_a is DMA+DVE only, phase_b is PE+ACT plus
        # trailing DVE accumulates, so: all phase_a work is emitted before
        # the phase_b that overlaps it; copies are interleaved with the
        # first chunk's m-blocks so ACT reaches the first exps as soon as
        # PSUM data exists; colsum matmuls are emitted two chunks late.
        issue_load(0)
        # The DMA engines fair-share across all queued descriptors, so
        # co-issuing H1 delays H0's quarters (and the first square pass) by
        # ~4us. Gate H1's load behind H0: a 1-element DVE copy into H1's
        # tile that reads the last H0 quarter makes the H1 load trigger
        # wait (write-after-write) until H0 has landed.
        nc.vector.tensor_copy(rows_t[1][:1, :1], rows_q0[3][:1, :1])
        issue_load(1)
        phase_a(0)
        emit_copies(0, 2)
        phase_b(0, 0, 2)
        emit_copies(2, 5)
        phase_b(0, 2, 5)
        emit_copies(5, MB)
        phase_b(0, 5, MB)
        phase_a(1)
        phase_a(2)
        phase_a(3)
        phase_a(4)
        phase_b(1, 0, MB)
        phase_b(2, 0, MB)
        emit_cs(1, 0)
        emit_cs(2, 2048)

        # --- outputs -------------------------------------------------------
        nc.sync.dma_start(out=cs_out[:, :], in_=cs_sb[:])
        nc.sync.dma_start(out=rs_out[:, :], in_=rs_all[:])
        nc.sync.dma_start(out=pos_out[:, :], in_=pos[:])

    nc.compile()
    return nc


_CACHE_LOCK = threading.Lock()
_CACHED_NC = None


def _get_nc():
    global _CACHED_NC
    with _CACHE_LOCK:
        if _CACHED_NC is None:
            _CACHED_NC = _build_kernel()
        return _CACHED_NC


def _run(inputs, trace=False):
    z_i = np.asarray(inputs["z_i"], dtype=np.float32)
    z_j = np.asarray(inputs["z_j"], dtype=np.float32)
    reps = np.concatenate([z_i, z_j], axis=0).astype(ml_dtypes.bfloat16)
    in_maps = [
        {"reps": np.ascontiguousarray(
            np.roll(reps, -ROWS_PER_CORE * i, axis=0)[:HH * HC])}
        for i in range(N_CORES)
    ]
    nc = _get_nc()
    res = run_bass_kernel_spmd(nc, in_maps, list(range(N_CORES)), trace=trace)

    # host epilogue: assemble denominators from partial rowsums + mirrored
    # colsums, then loss = mean(ln(denom) - ln(e_pos)).
    q = np.arange(HC)
    perm = (q % TPH) * P + q // TPH  # permuted col q -> row within half
    denom = np.zeros(TWO_N, dtype=np.float64)
    posv = np.zeros(TWO_N, dtype=np.float64)
    pp, mm = np.meshgrid(np.arange(P), np.arange(MB), indexing="ij")
    for i in range(N_CORES):
        base = ROWS_PER_CORE * i
        r = res.results[i]
        rs = np.asarray(r["rs_out"], dtype=np.float64)      # [128, MB*NSLOT]
        po = np.asarray(r["pos_out"], dtype=np.float64)     # [128, MB]
        csv = np.asarray(r["cs_out"], dtype=np.float64)     # [1, 3072]
        rows = (base + mm * P + pp) % TWO_N                 # [128, MB]
        rowsum = rs.reshape(P, MB, NSLOT).sum(axis=2)
        denom[rows] += rowsum - E_SELF
        posv[rows] = po
        for c in range(6):
            hh = 1 + c // 2
            qh = (c % 2) * 512 + np.arange(512)
            cols = (base + hh * HC + perm[qh]) % TWO_N
            denom[cols] += csv[0, c * 512:(c + 1) * 512]
    loss = (np.log(denom).sum() - np.log(posv).sum()) / TWO_N
    return np.float32(loss), res


def kernel(**inputs):
    loss, _ = _run(inputs, trace=False)
    return np.asarray(loss, dtype=np.float32)


# revision 52
# speedup vs baseline: 1.0423x; 1.0423x over previous
"""NT-Xent / SimCLR contrastive loss on 8 Trainium2 NeuronCores.

Symmetry-halved data-parallel strategy:
  sim is symmetric, so each unordered row pair only needs one on-device
  exp(). Core i (inputs rolled by -1024*i rows) computes its 1024 rows
  against rolled columns [0, 5120) only:
    d0 = cols [0,1024):    self block (complete rowsums, fully covers its
                           own pairs - both (r,c) and (c,r) are inside).
    d1-d3 = [1024,4096):   each pair {i,i+d} computed ONCE (by core i);
                           core i+d's missing mirror contributions are
                           returned as per-column sums of exp (colsums) and
                           added on the host.
    d4 = [4096,5120):      pair {i,i+4} computed by BOTH cores (cheaper
                           than exchanging mirrors); positives live on this
                           block's diagonal.
  Device pipeline per core (fp8 DoubleRow, one kernel launch):
    phase A (per 1024-row half): load bf16 rows; 1/||row|| via fused
      square+rowsum then a Quake-style bitwise rsqrt + 2 Newton steps (all
      DVE - keeps ACT free and avoids cross-queue stalls); scale rows into
      permuted fp8e4 staging; DRAM scratch store + xbar transpose-load into
      [128,1024] u16 repsT8 tiles (u16 = the 2 DoubleRow fp8 planes).
    phase B (chunk-outer, m-inner; chunks [1024,1536,1536,1024] wide):
      sim chunk via DR fp8 matmuls (kk-outer so LDWEIGHTS amortizes over
      the chunk) -> PSUM [128,<=1536]; ACT exp(2/C^2 * sim) with fused
      row-sum accumulate; colsums of the d1-d3 exp tiles via ones-onehot
      matmuls accumulated over m in a [8,512] PSUM bank; positives diag
      extracted from the d4 chunk with an identity-mask STT (DVE).
    e_self is NOT extracted: sim_ii = 1 +- ~0.4% in fp8, so the host
    subtracts exp(2) exactly - error ~1e-6 relative on the loss.
  Host: assemble denom[8192] from per-core partial rowsums + mirrored
  colsums (undoing the transpose permutation), then
  loss = mean(ln(denom) - ln(e_pos)).
"""

import math
import sys
import threading
from unittest import mock

sys.path.insert(0, "/opt/trn_rl_repo")

import numpy as np  # noqa: E402
import ml_dtypes  # noqa: E402

import concourse.tile as tile  # noqa: E402
from concourse import bacc, mybir  # noqa: E402
from concourse.bass_utils import run_bass_kernel_spmd  # noqa: E402
from concourse.hw_specs import get_activation_tables  # noqa: E402
from concourse.masks import make_identity  # noqa: E402
from contextlib import ExitStack  # noqa: E402

P = 128
D = 512
TWO_N = 8192
N_CORES = 8
ROWS_PER_CORE = TWO_N // N_CORES  # 1024
T_INV = 2.0  # 1 / temperature (0.5)

HH = 5  # 1024-row halves normalized/transposed per core (cols [0, 5120))
HC = 1024  # columns per half
TPH = HC // P  # 8 [128, 512] row tiles per half
MB = ROWS_PER_CORE // P  # 8 m-blocks of 128 rows per core
KK = 2  # DoubleRow packed K chunks (256 features each)
NSLOT = 3  # phase-B chunks per m (rowsum slots)
PSW = 2048  # PSUM sim tile width (4 banks x 2 bufs = all 8; colsums ride
            # the same ring as disjoint [1,512] slices)
MAGIC = 0x5F3759DF  # fast inverse sqrt

C_SCALE = 512.0  # fp8 range scale; rows stored as C*x/||x||
SIM_SCALE = T_INV / (C_SCALE * C_SCALE)  # exact power of two: 2^-17
E_SELF = float(np.exp(np.float64(T_INV)))  # host-subtracted self term

# phase-B chunks: (col0, width). Chunk 0 only needs half 0 (short serial
# prefix); positives (d4) sit in chunk 2's second half.
CHUNKS = [
    (0, 1024),     # d0 (self block)
    (1024, 2048),  # d1 + d2
    (3072, 2048),  # d3 + d4
]
# colsum classes: chunk 1 contributes 4 512-wide pairs (d1a,b d2a,b),
# chunk 2 contributes 2 (d3a,b; d4 needs no mirror). Host mapping of pair
# c: half 1 + c//2, within-half columns (c%2)*512 ..
ACC_W = {1: 2048, 2: 1024}  # colsum-relevant width of each chunk's et

FP32 = mybir.dt.float32
BF16 = mybir.dt.bfloat16
FP8 = mybir.dt.float8e4
U16 = mybir.dt.uint16
I32 = mybir.dt.int32
AF = mybir.ActivationFunctionType
ALU = mybir.AluOpType
AX = mybir.AxisListType
DR = mybir.MatmulPerfMode.DoubleRow


def _filtered_activation_tables(arch):
    """Steer every Exp/Ln/Copy activation to the one table set containing
    both Exp and Ln, so the table-load pass cannot thrash between sets."""
    tables = get_activation_tables(arch)
    target = None
    for name, funcs in tables.items():
        if AF.Exp in funcs and AF.Ln in funcs:
            target = name
            break
    if target is None:
        return tables
    steer = {AF.Exp, AF.Ln, AF.Copy, AF.Identity}
    return {
        name: (funcs if name == target else funcs - steer)
        for name, funcs in tables.items()
    }


def _build_kernel():
    nc = bacc.Bacc("TRN2", target_bir_lowering=False, debug=False,
                   num_devices=N_CORES)
    reps = nc.dram_tensor("reps", [HH * HC, D], BF16,
                          kind="ExternalInput").ap()
    rs_out = nc.dram_tensor("rs_out", [P, MB * NSLOT], FP32,
                            kind="ExternalOutput").ap()
    pos_out = nc.dram_tensor("pos_out", [P, MB], FP32,
                             kind="ExternalOutput").ap()
    cs_out = nc.dram_tensor("cs_out", [1, 3072], FP32,
                            kind="ExternalOutput").ap()

    with tile.TileContext(nc) as tc, ExitStack() as ctx:
        rows_pool = ctx.enter_context(tc.tile_pool(name="rows", bufs=1))
        stag_pool = ctx.enter_context(tc.tile_pool(name="stag", bufs=3))
        sq_pool = ctx.enter_context(tc.tile_pool(name="sq", bufs=2))
        stats_pool = ctx.enter_context(tc.tile_pool(name="stats", bufs=1))
        repsT_pool = ctx.enter_context(tc.tile_pool(name="repsT", bufs=1))
        repsT0_pool = ctx.enter_context(tc.tile_pool(name="repsT0", bufs=1))
        dram_pool = ctx.enter_context(
            tc.tile_pool(name="scratch", bufs=KK * HH, space="DRAM"))
        psum_pool = ctx.enter_context(
            tc.tile_pool(name="psum", bufs=2, space="PSUM"))
        exp_pool = ctx.enter_context(tc.tile_pool(name="exp", bufs=10))
        junk_pool = ctx.enter_context(tc.tile_pool(name="junk", bufs=2))
        epi_pool = ctx.enter_context(tc.tile_pool(name="epi", bufs=1))

        # --- constants -----------------------------------------------------
        ident = stats_pool.tile([P, P], FP32, tag="ident", name="ident")
        make_identity(nc, ident[:])
        ones = stats_pool.tile([P, 1], BF16, tag="ones", name="ones")
        nc.gpsimd.memset(ones[:], 1.0)

        # accumulators
        rs_all = stats_pool.tile([P, MB * NSLOT], FP32, tag="rs",
                                 name="rs_all")
        pos = stats_pool.tile([P, MB], FP32, tag="pos", name="pos")
        cs_sb = epi_pool.tile([1, 3072], FP32, tag="cssb", name="cs_sb")

        # repsT8[kk][hh]: [128, 1024] u16 - partition p = feature pair
        # (kk*256 + 2p, 2p+1) packed as 2 fp8 bytes; column q = half row
        # (q%8)*128 + q//8 (scratch permutation). fp8 bitcast views give
        # the DoubleRow [128, 2, N] moving-operand APs directly.
        repsT8 = [[repsT_pool.tile([P, HC], U16, tag=f"rT{kk}_{hh}",
                                   name=f"repsT8_{kk}_{hh}")
                   for hh in range(HH)]
                  for kk in range(KK)]

        def rhs_ap(kk, hh, sub):
            v = repsT8[kk][hh][:].bitcast(FP8).rearrange(
                "p (n two) -> p two n", two=2)
            return v[:, :, sub * 512:(sub + 1) * 512]

        # repsT0[kk]: [128, 2*1024] fp8, plane-slab layout - LDWEIGHTS
        # rejects the byte-interleaved stride-2 AP, so the core's own 1024
        # columns are deinterleaved for the stationary side.
        repsT0 = [repsT0_pool.tile([P, 2 * ROWS_PER_CORE], FP8,
                                   tag=f"rT0_{kk}", name=f"repsT0_{kk}")
                  for kk in range(KK)]

        def lhs_ap(kk, m):
            v = repsT0[kk][:].rearrange("p (two m) -> p two m", two=2)
            return v[:, :, m * P:(m + 1) * P]

        # --- loads ---------------------------------------------------------
        # half 0 lands in four quarter tiles so the first square pass only
        # waits for 256 rows (dependency tracking is tile-granular).
        rows_q0 = [rows_pool.tile([P, 2 * D], BF16, tag=f"rows0q{qq}",
                                  name=f"rows_0q{qq}") for qq in range(4)]
        rows_t = [None] + [rows_pool.tile([P, TPH * D], BF16,
                                          tag=f"rows{hh}", name=f"rows_{hh}")
                           for hh in range(1, HH)]

        def row_slice(hh, tl):
            if hh == 0:
                return rows_q0[tl // 2][:, (tl % 2) * D:(tl % 2 + 1) * D]
            return rows_t[hh][:, tl * D:(tl + 1) * D]

        def issue_load(hh):
            if hh == 0:
                for qq in range(4):
                    r0 = qq * 2 * P
                    src = reps[r0:r0 + 2 * P, :].rearrange(
                        "(t p) d -> p t d", p=P)
                    nc.sync.dma_start(
                        out=rows_q0[qq][:].rearrange("p (t d) -> p t d", d=D),
                        in_=src)
                return
            for s in range(2):
                r0 = hh * HC + s * (TPH // 2) * P
                src = reps[r0:r0 + (TPH // 2) * P, :].rearrange(
                    "(t p) d -> p t d", p=P)
                dst = rows_t[hh][:, s * (TPH // 2) * D:(s + 1) * (TPH // 2) *
                                 D].rearrange("p (t d) -> p t d", d=D)
                nc.sync.dma_start(out=dst, in_=src)

        def phase_a(hh):
            n2 = stats_pool.tile([P, TPH], FP32, tag="n2", bufs=3,
                                 name=f"n2_{hh}")
            for tl in range(TPH):
                sq = sq_pool.tile([P, D], BF16, tag="sq",
                                  name=f"sq_{hh}_{tl}")
                rt = row_slice(hh, tl)
                nc.vector.scalar_tensor_tensor(
                    out=sq[:], in0=rt, scalar=1.0, in1=rt,
                    op0=ALU.mult, op1=ALU.mult,
                    accum_out=n2[:, tl:tl + 1])
            # inv = C * rsqrt(n2), entirely on DVE (bit trick + 1 Newton
            # step, max rel err ~1.6e-3 on inv -> ~1e-4 on the loss): keeps
            # ACT for exps/copies and avoids cross-queue ping-pong stalls.
            # (Tried computing half 0's inv on the prefix-idle ACT via
            # ln/exp instead: consistently ~3us SLOWER end-to-end.)
            w = stats_pool.tile([P, 4 * TPH], FP32, tag="rsq", bufs=3,
                                name=f"rsq_{hh}")
            t_i, y_i = w[:, 0:TPH].bitcast(I32), w[:, TPH:2 * TPH]
            aa, cc = w[:, 2 * TPH:3 * TPH], w[:, 3 * TPH:4 * TPH]
            inv = stats_pool.tile([P, TPH], FP32, tag="inv", bufs=3,
                                  name=f"inv_{hh}")
            nc.vector.tensor_scalar(out=t_i, in0=n2[:].bitcast(I32),
                                    scalar1=1, scalar2=None,
                                    op0=ALU.logical_shift_right)
            nc.vector.tensor_scalar(out=y_i.bitcast(I32), in0=t_i,
                                    scalar1=MAGIC, scalar2=-1,
                                    op0=ALU.subtract, op1=ALU.mult)
            nc.vector.tensor_mul(aa, y_i, y_i)       # y0^2
            nc.vector.scalar_tensor_tensor(
                out=cc, in0=aa, scalar=-0.5, in1=n2[:],
                op0=ALU.mult, op1=ALU.mult)          # -0.5*n2*y0^2
            nc.vector.tensor_scalar(out=aa, in0=cc, scalar1=1.5,
                                    scalar2=C_SCALE, op0=ALU.add,
                                    op1=ALU.mult)    # C*(1.5 - n2*y0^2/2)
            nc.vector.tensor_mul(inv[:], y_i, aa)    # C * rsqrt(n2)
            # permuted fp8 staging: byte addr = kk*2048 + t*256 + c so each
            # kk store is one contiguous 2 KiB run per partition.
            stag = stag_pool.tile([P, KK * TPH * 256], FP8, tag="stag",
                                  name=f"stag_{hh}")
            sview = stag[:].rearrange("p (kk t c) -> p t kk c",
                                      kk=KK, c=256)
            for tl in range(TPH):
                src3 = row_slice(hh, tl).rearrange(
                    "p (kk c) -> p kk c", kk=KK)
                nc.vector.tensor_scalar_mul(
                    sview[:, tl], src3, inv[:, tl:tl + 1])
            # scratch row q = p*8 + t holds half row t*128 + p; the
            # transpose-load is then fully contiguous.
            for kk in range(KK):
                scr = dram_pool.tile([HC, P], U16, tag=f"scr{kk}_{hh}",
                                     name=f"scr_{kk}_{hh}")
                src = stag[:, kk * TPH * 256:(kk + 1) * TPH * 256]
                nc.sync.dma_start(
                    out=scr[:].rearrange("(p t) c -> p t c", p=P),
                    in_=src.bitcast(U16).rearrange("p (t c) -> p t c", c=P))
                nc.sync.dma_start_transpose(repsT8[kk][hh][:], scr[:])
            # prefetch AFTER this half's stores/transposes: the sync DMA
            # queue is in-order, so loads issued earlier would delay them.
            if hh + 2 < HH:
                issue_load(hh + 2)

        # permuted column q of an et tile (within a half) holds half row
        # (q%8)*128 + q//8, so the columns for m-block rows m*128+j sit at
        # positions 8*j + m.
        def colsel(ap_1024, m):
            return ap_1024.rearrange("p (j s) -> p s j", s=TPH)[:, m, :]

        def emit_copies(s0, s1):
            # deinterleave + unpermute the core's own 1024 columns into
            # plane-slab lhsT (fp8 index 2*(8j+s)+i for half row s*128+j).
            # ACT does these strided copies (~0.7-1.6us each; DVE/GPSIMD
            # measured ~4us on the same APs).
            for kk in range(KK):
                iv = repsT8[kk][0][:].bitcast(FP8).rearrange(
                    "p (j s two) -> p two s j", two=2, s=TPH)
                ov = repsT0[kk][:].rearrange(
                    "p (two s m) -> p two s m", two=2, s=MB)
                for i in range(2):
                    nc.scalar.activation(
                        ov[:, i, s0:s1], iv[:, i, s0:s1], AF.Copy)

        # colsum accumulators: acc[ci] = sum over m of et tiles, built by
        # DVE bf16 adds in its post-phase-A idle window; the PE then only
        # streams one ones-matmul per 512-wide pair instead of eight.
        acc_t = {ci: stats_pool.tile([P, ACC_W[ci]], BF16, tag=f"acc{ci}",
                                     name=f"acc_{ci}")
                 for ci in ACC_W}

        def phase_b(ci, m0, m1):
            col0, w = CHUNKS[ci]
            subs = [((col0 + 512 * s) // HC, ((col0 + 512 * s) % HC) // 512)
                    for s in range(w // 512)]
            for m in range(m0, m1):
                ps = psum_pool.tile([P, PSW], FP32, tag="ps",
                                    name=f"ps_{ci}_{m}")
                for kk in range(KK):
                    for idx, (hh, sub) in enumerate(subs):
                        nc.tensor.matmul(
                            ps[:, idx * 512:(idx + 1) * 512],
                            lhsT=lhs_ap(kk, m),
                            rhs=rhs_ap(kk, hh, sub),
                            start=(kk == 0), stop=(kk == KK - 1),
                            perf_mode=DR)
                et = exp_pool.tile([P, PSW], BF16, tag="et",
                                   name=f"et_{ci}_{m}")
                sl = m * NSLOT + ci
                nc.scalar.activation(
                    et[:, :w], ps[:, :w], AF.Exp, scale=SIM_SCALE,
                    accum_out=rs_all[:, sl:sl + 1])
                if ci in acc_t:
                    a = acc_t[ci]
                    aw = ACC_W[ci]
                    if m == 0:
                        nc.vector.tensor_copy(a[:], et[:, :aw])
                    else:
                        nc.vector.tensor_add(a[:], a[:], et[:, :aw])
                if ci == 2:
                    # positives: global col 4096 + (m*128+j) at permuted
                    # position 8j+m of the d4 half (= this chunk's second
                    # half). exp(2*s_pos) read from et; ln on the host.
                    junk = junk_pool.tile([P, P], FP32, tag="junk",
                                          name=f"junk_p_{m}")
                    nc.vector.scalar_tensor_tensor(
                        out=junk[:], in0=colsel(et[:, HC:2 * HC], m),
                        scalar=1.0, in1=ident[:],
                        op0=ALU.mult, op1=ALU.mult,
                        accum_out=pos[:, m:m + 1])

        def emit_cs(ci, sb_off):
            # colsums land in disjoint [1,512] slices of one ps-ring tile
            # (ones-lhsT matmuls), then a DVE drain into cs_sb. Emitted
            # late enough that the PE reaches these only after the DVE
            # finished summing the class's et tiles.
            aw = ACC_W[ci]
            pst = psum_pool.tile([P, PSW], FP32, tag="ps",
                                 name=f"cs_ps_{ci}")
            for s in range(aw // 512):
                nc.tensor.matmul(
                    pst[:1, s * 512:(s + 1) * 512], lhsT=ones[:],
                    rhs=acc_t[ci][:, s * 512:(s + 1) * 512],
                    start=True, stop=True)
            if ci == 2:
                # the final drain is on the tail critical path: ACT is done
                # by then while the DVE still has the last pos-diag STT, so
                # run it on ACT to overlap.
                nc.scalar.activation(cs_sb[:, sb_off:sb_off + aw],
                                     pst[:1, :aw], AF.Copy)
            else:
                nc.vector.tensor_copy(cs_sb[:, sb_off:sb_off + aw],
                                      pst[:1, :aw])

        # --- pipeline ------------------------------------------------------
        # Emission order = per-engine queue order (queues execute in order,
        # so anything emitted early that waits on a late dependency blocks
        # its whole queue). phase_a is DMA+DVE only, phase_b is PE+ACT plus
        # trailing DVE accumulates, so: all phase_a work is emitted before
        # the phase_b that overlaps it; copies are interleaved with the
        # first chunk's m-blocks so ACT reaches the first exps as soon as
        # PSUM data exists; colsum matmuls are emitted two chunks late.
        issue_load(0)
        issue_load(1)
        phase_a(0)
        emit_copies(0, 2)
        phase_b(0, 0, 2)
        emit_copies(2, 5)
        phase_b(0, 2, 5)
        emit_copies(5, MB)
        phase_b(0, 5, MB)
        phase_a(1)
        phase_a(2)
        phase_a(3)
        phase_a(4)
        phase_b(1, 0, MB)
        phase_b(2, 0, MB)
        emit_cs(1, 0)
        emit_cs(2, 2048)

        # --- outputs -------------------------------------------------------
        nc.sync.dma_start(out=cs_out[:, :], in_=cs_sb[:])
        nc.sync.dma_start(out=rs_out[:, :], in_=rs_all[:])
        nc.sync.dma_start(out=pos_out[:, :], in_=pos[:])

    nc.compile()
    return nc


_CACHE_LOCK = threading.Lock()
_CACHED_NC = None


def _get_nc():
    global _CACHED_NC
    with _CACHE_LOCK:
        if _CACHED_NC is None:
            _CACHED_NC = _build_kernel()
        return _CACHED_NC


def _run(inputs, trace=False):
    z_i = np.asarray(inputs["z_i"], dtype=np.float32)
    z_j = np.asarray(inputs["z_j"], dtype=np.float32)
    reps = np.concatenate([z_i, z_j], axis=0).astype(ml_dtypes.bfloat16)
    in_maps = [
        {"reps": np.ascontiguousarray(
            np.roll(reps, -ROWS_PER_CORE * i, axis=0)[:HH * HC])}
        for i in range(N_CORES)
    ]
    nc = _get_nc()
    res = run_bass_kernel_spmd(nc, in_maps, list(range(N_CORES)), trace=trace)

    # host epilogue: assemble denominators from partial rowsums + mirrored
    # colsums, then loss = mean(ln(denom) - ln(e_pos)).
    q = np.arange(HC)
    perm = (q % TPH) * P + q // TPH  # permuted col q -> row within half
    denom = np.zeros(TWO_N, dtype=np.float64)
    posv = np.zeros(TWO_N, dtype=np.float64)
    pp, mm = np.meshgrid(np.arange(P), np.arange(MB), indexing="ij")
    for i in range(N_CORES):
        base = ROWS_PER_CORE * i
        r = res.results[i]
        rs = np.asarray(r["rs_out"], dtype=np.float64)      # [128, MB*NSLOT]
        po = np.asarray(r["pos_out"], dtype=np.float64)     # [128, MB]
        csv = np.asarray(r["cs_out"], dtype=np.float64)     # [1, 3072]
        rows = (base + mm * P + pp) % TWO_N                 # [128, MB]
        rowsum = rs.reshape(P, MB, NSLOT).sum(axis=2)
        denom[rows] += rowsum - E_SELF
        posv[rows] = po
        for c in range(6):
            hh = 1 + c // 2
            qh = (c % 2) * 512 + np.arange(512)
            cols = (base + hh * HC + perm[qh]) % TWO_N
            denom[cols] += csv[0, c * 512:(c + 1) * 512]
    loss = (np.log(denom).sum() - np.log(posv).sum()) / TWO_N
    return np.float32(loss), res


def kernel(**inputs):
    loss, _ = _run(inputs, trace=False)
    return np.asarray(loss, dtype=np.float32)
